# revision 5
# baseline (speedup 1.0000x reference)
"""Trainium2 Bass kernel for nn_AttnBlock (per-pixel qk attention block).

Reference computation (per batch b):
  q = x @ wq.T ; k = x @ wk.T ; v = x @ wv.T          # [H*W, 512], heads n=8, d=64
  s[n, p]    = sum_d q[p, n*64+d] * k[p, n*64+d]      # per-pixel dot product
  w[n, h, :] = softmax(s[n, h, :] * d**-0.5)          # softmax over W axis (32)
  vsum[n, p] = sum_d v[p, n*64+d]
  out[b, n, hw, xy] = w[n, hw] * vsum[n, xy]          # outer product per batch

Sharding: data-parallel over batch: core b handles batch b (8 cores, B=8).

The kernel is output-write bound: fp16 output (host upcasts; rel err ~1e-3
vs the 2e-2 gate) => 16 MB/core written + ~2.4 MB read at the ~360 GB/s
per-NC DMA limit => ~52.5 us DMA floor. Total = t_first_output + 46.6 us
(gapless output stream) + ~1.6 us DMA-sem/barrier tail, so the design
minimizes t_first_output and keeps the output queue fed:

- x^T is loaded in pixel halves (8 DMAs) so pair-0 q/k matmuls start
  after 0.5 MB of x; the first output block (head 0, rows 0:4, xy 0:512)
  depends only on the first x half (vsum half too).
- scores are computed TRANSPOSED: 4 tiny matmuls sprod_chunk^T @ ind2
  -> [128px, 2 heads] PSUM, so softmax weights come out pixel-partitioned
  with no PE transposes. Denominators: ones-blockdiag32^T @ e -> [4, 8],
  reciprocal, then sel32^T broadcast back to [128, 8]; w = e * rden_bcast.
- y DRAM layout is [n, p, c, xy] (hw = c*128 + p): per-partition
  contiguous (c, xy) runs give 4-16 KB DMA descriptors (vs 2 KB with
  [n, hw, xy]); host un-permutes (layout only).
- PE warm-up: dummy matmuls (own PSUM bank, emitted last = lowest
  priority) keep the tensor engine continuously busy from t~0 so it
  reaches full clock (2.4 GHz after 3 us busy) before the x-gated burst;
  narrow (N=256) so each mid-kernel steal costs ~107 ns.
- k's PSUM never drains: sprod = q*k reads k's PSUM bank directly (walrus
  allows one PSUM input per TensorTensor); q drains on ACT (DVE for
  pair-0 half-0 where DVE is otherwise idle). Softmax in fp16, no
  max-subtraction (logits ~N(0, 0.33)).
- production per head: PE selector-matmul broadcast of vsum halves
  (drains on ACT), outer-product tiles via DVE tensor_scalar fp16
  (4x mode; 2 of 8 rows on ACT for pairs 1-3).
"""

import numpy as np

import concourse.bass as bass
import concourse.mybir as mybir
import concourse.tile as tile
from concourse import bacc
from concourse.bass_utils import run_bass_kernel_spmd

F32 = mybir.dt.float32
F16 = mybir.dt.float16

B, HW, DIM = 8, 1024, 512
N_HEADS, D_HEAD = 8, 64
N_CORES = 8
SCALE = float(D_HEAD) ** -0.5

QK_DT = F16
QK_NP = np.float16
OUT_DT = F16

N_WARMUP = 36  # dummy PE matmuls: ramp + keep-hot filler for PE gaps


def build_program(loop_iters=None):
    """loop_iters: if set, wrap the whole kernel body in a tc.For_i hardware
    loop (benchmarking only -- one NEFF executes the body N times)."""
    # Bacc (not raw Bass): its compile() runs move_matmul_waits_to_ldweights,
    # without which any matmul with >1 semaphore wait fails walrus codegen.
    nc = bacc.Bacc(None)

    xt_d = nc.declare_dram_parameter("xt", [DIM, HW], QK_DT, isOutput=False)
    # pre = wv_sum, ind2, blockdiag32, and the pair-0 wq/wk slices: one DMA
    pre_d = nc.declare_dram_parameter("pre", [128, 1062], QK_DT, isOutput=False)
    w1_d = nc.declare_dram_parameter("w1", [128, 2, 4, 128], QK_DT,
                                     isOutput=False)
    w23_d = nc.declare_dram_parameter(
        "w23", [128, 2, 2, 4, 128], QK_DT, isOutput=False
    )
    aux2_d = nc.declare_dram_parameter("aux2", [8, 1152], QK_DT, isOutput=False)
    # y layout [n, p, c, xy] (hw = c*128 + p); host un-permutes.
    y_d = nc.declare_dram_parameter("y", [N_HEADS, 128, 8, HW], OUT_DT,
                                    isOutput=True)

    with tile.TileContext(nc) as tc:
        with (
            tc.tile_pool(name="singles", bufs=1) as singles,
            tc.tile_pool(name="sprod", bufs=2) as sprodp,
            tc.tile_pool(name="smax", bufs=2) as smaxp,
            tc.tile_pool(name="wt", bufs=4) as wtp,
            tc.tile_pool(name="bc", bufs=4) as bcp,
            tc.tile_pool(name="prod", bufs=2) as prodp,
            tc.tile_pool(name="warm_ps", bufs=1, space="PSUM") as warm_ps,
            tc.tile_pool(name="qk_ps", bufs=3, space="PSUM") as qk_ps,
            tc.tile_pool(name="s_ps", bufs=2, space="PSUM") as s_ps,
            tc.tile_pool(name="v_ps", bufs=1, space="PSUM") as v_ps,
        ):
            def emit_body():
                # ---- PE warm-up tile (dummy matmuls emitted last, so they
                # have the lowest priority and only fill idle PE slots) ------
                wm = singles.tile([128, 256], QK_DT, name="wm")
                nc.gpsimd.memset(wm, 0.0)

                # ---- loads: one FIFO queue (sync HWDGE), priority order ----
                pre_sb = singles.tile([128, 1062], QK_DT)
                nc.sync.dma_start(out=pre_sb, in_=pre_d[:])
                wvt_sb = pre_sb[:, 0:32].rearrange("p (k n) -> p k n", k=4)
                ind2_sb = pre_sb[:, 32:34]
                bd32_sb = pre_sb[:, 34:38]
                wq0_sb = pre_sb[:, 38:550].rearrange("p (k o) -> p k o", k=4)
                wk0_sb = pre_sb[:, 550:1062].rearrange("p (k o) -> p k o", k=4)

                xT = [
                    singles.tile([128, HW], QK_DT, name=f"xT{ki}")
                    for ki in range(4)
                ]
                xv = xt_d[:].rearrange("(k p) xy -> p k xy", p=128)
                for ki in range(4):  # pixel half 0 first
                    nc.sync.dma_start(
                        out=xT[ki][:, 0:512], in_=xv[:, ki, 0:512]
                    )
                aux2_sb = singles.tile([8, 1152], QK_DT)
                nc.sync.dma_start(out=aux2_sb, in_=aux2_d[:])
                sel_sb = aux2_sb[:, 0:1024]
                sel32_sb = aux2_sb[0:4, 1024:1152]
                for ki in range(4):  # pixel half 1
                    nc.sync.dma_start(
                        out=xT[ki][:, 512:1024], in_=xv[:, ki, 512:1024]
                    )
                w1_sb = singles.tile([128, 2, 4, 128], QK_DT)   # pair 1
                nc.sync.dma_start(out=w1_sb, in_=w1_d[:])
                w23_sb = singles.tile([128, 2, 2, 4, 128], QK_DT)
                nc.sync.dma_start(out=w23_sb, in_=w23_d[:])

                vps = v_ps.tile([N_HEADS, HW], F32)
                vsum_sb = singles.tile([N_HEADS, HW], QK_DT)

                def wslice(qk, ti, ki):
                    if ti == 0:
                        return (wq0_sb if qk == 0 else wk0_sb)[:, ki, :]
                    if ti == 1:
                        return w1_sb[:, qk, ki, :]
                    return w23_sb[:, qk, ti - 2, ki, :]

                def emit_qk_half(ti, nj):
                    """q/k matmul groups for pixel-half nj of pair ti."""
                    qps = qk_ps.tile([128, 512], F32, tag="qk", name="qps")
                    kps = qk_ps.tile([128, 512], F32, tag="qk", name="kps")
                    for qk, ps in ((0, qps), (1, kps)):
                        for ki in range(4):
                            nc.tensor.matmul(
                                ps,
                                wslice(qk, ti, ki),
                                xT[ki][:, nj * 512 : (nj + 1) * 512],
                                start=(ki == 0),
                                stop=(ki == 3),
                            )
                    return qps, kps

                def emit_vsum_half(nj):
                    csl = slice(nj * 512, (nj + 1) * 512)
                    for ki in range(4):
                        nc.tensor.matmul(
                            vps[:, csl],
                            wvt_sb[:, ki, :],
                            xT[ki][:, csl],
                            start=(ki == 0),
                            stop=(ki == 3),
                        )
                    nc.scalar.copy(vsum_sb[:, csl], vps[:, csl])

                def emit_scores_half(qps, kps, q_on_dve=False):
                    """Transposed scores: w_t[p, c*2+j] = softmax weight of
                    head j at pixel c*128+p (within the half). q drains to
                    SBUF (ACT; DVE for pair-0 h0); sprod reads k straight
                    from its PSUM bank (<=1 PSUM input per TensorTensor)."""
                    qt = sprodp.tile([128, 512], QK_DT, tag="qt")
                    if q_on_dve:
                        nc.vector.tensor_copy(qt, qps)
                    else:
                        nc.scalar.copy(qt, qps)
                    sprod = sprodp.tile([128, 512], QK_DT, tag="sp")
                    nc.vector.tensor_tensor(
                        out=sprod, in0=qt, in1=kps, op=mybir.AluOpType.mult,
                    )
                    sp = s_ps.tile([128, 24], F32, tag="s", name="sp")
                    for c in range(4):
                        nc.tensor.matmul(
                            sp[:, 2 * c : 2 * c + 2],
                            sprod[:, c * 128 : (c + 1) * 128],
                            ind2_sb,
                            start=True, stop=True,
                        )
                    e_t = smaxp.tile([128, 8], QK_DT, tag="e")
                    nc.scalar.activation(
                        out=e_t, in_=sp[:, 0:8],
                        func=mybir.ActivationFunctionType.Exp,
                        scale=SCALE,
                    )
                    nc.tensor.matmul(
                        sp[0:4, 8:16], bd32_sb, e_t, start=True, stop=True
                    )
                    rden = smaxp.tile([4, 8], QK_DT, tag="r")
                    with nc.allow_low_precision(reason="fp16 softmax denom"):
                        nc.vector.reciprocal(rden, sp[0:4, 8:16])
                    nc.tensor.matmul(
                        sp[:, 16:24], sel32_sb, rden, start=True, stop=True
                    )
                    w_t = wtp.tile([128, 8], QK_DT, tag="w", name="wt")
                    nc.vector.tensor_tensor(
                        out=w_t, in0=e_t, in1=sp[:, 16:24],
                        op=mybir.AluOpType.mult,
                    )
                    return w_t

                def emit_bcast_half(head, nj, bcast_t):
                    """vsum half row -> all partitions via PE selector
                    matmul; PSUM drains on ACT into bcast_t's half."""
                    csl = slice(nj * 512, (nj + 1) * 512)
                    bps = qk_ps.tile([128, 512], F32, tag="qk", name="bps")
                    nc.tensor.matmul(
                        bps,
                        sel_sb[:, head * 128 : (head + 1) * 128],
                        vsum_sb[:, csl],
                        start=True, stop=True,
                    )
                    nc.scalar.copy(bcast_t[:, csl], bps)

                def produce(prod_t, j, cj, w_half, bcast_t, xy0, xy1,
                            use_act=False):
                    col = (cj % 4) * 2 + j
                    if use_act:
                        nc.scalar.activation(
                            out=prod_t[:, j, cj, xy0:xy1],
                            in_=bcast_t[:, xy0:xy1],
                            func=mybir.ActivationFunctionType.Copy,
                            scale=w_half[:, col : col + 1],
                        )
                    else:
                        nc.vector.tensor_scalar_mul(
                            prod_t[:, j, cj, xy0:xy1],
                            bcast_t[:, xy0:xy1],
                            w_half[:, col : col + 1],
                        )

                def dma_block(head, prod_t, j, c0, c1, xy0, xy1):
                    nc.sync.dma_start(
                        out=y_d[head : head + 1].rearrange(
                            "n p c xy -> p n c xy"
                        )[:, :, c0:c1, xy0:xy1],
                        in_=prod_t[:, j : j + 1, c0:c1, xy0:xy1],
                    )

                # ---- pair 0: the first-output chain gets high priority ----
                with tc.high_priority():
                    qk00 = emit_qk_half(0, 0)
                    emit_vsum_half(0)
                    w00 = emit_scores_half(*qk00, q_on_dve=True)
                    prod0 = prodp.tile(
                        [128, 2, 8, HW], OUT_DT, tag="pr", name="pr"
                    )
                    bc0 = bcp.tile([128, HW], QK_DT, tag="bc", name="bc")
                    emit_bcast_half(0, 0, bc0)
                    for c in range(4):
                        produce(prod0, 0, c, w00, bc0, 0, 512)
                    dma_block(0, prod0, 0, 0, 4, 0, 512)
                    bc1 = bcp.tile([128, HW], QK_DT, tag="bc", name="bc")
                    emit_bcast_half(1, 0, bc1)
                    for c in range(4):
                        produce(prod0, 1, c, w00, bc1, 0, 512)
                    dma_block(1, prod0, 1, 0, 4, 0, 512)

                # pair 0, half 1 + remaining pair-0 blocks
                qk01 = emit_qk_half(0, 1)
                emit_vsum_half(1)
                w01 = emit_scores_half(*qk01)
                emit_bcast_half(0, 1, bc0)
                for c in range(4):
                    produce(prod0, 0, c, w00, bc0, 512, 1024)
                dma_block(0, prod0, 0, 0, 4, 512, 1024)
                emit_bcast_half(1, 1, bc1)
                for c in range(4):
                    produce(prod0, 1, c, w00, bc1, 512, 1024)
                dma_block(1, prod0, 1, 0, 4, 512, 1024)
                for c in range(4, 8):
                    produce(prod0, 0, c, w01, bc0, 0, 1024)
                dma_block(0, prod0, 0, 4, 8, 0, 1024)
                for c in range(4, 8):
                    produce(prod0, 1, c, w01, bc1, 0, 1024)
                dma_block(1, prod0, 1, 4, 8, 0, 1024)

                # ---- pairs 1-3 ----
                for ti in range(1, 4):
                    qps, kps = emit_qk_half(ti, 0)
                    w_h0 = emit_scores_half(qps, kps)
                    qps, kps = emit_qk_half(ti, 1)
                    w_h1 = emit_scores_half(qps, kps)
                    prod_t = prodp.tile(
                        [128, 2, 8, HW], OUT_DT, tag="pr", name="pr"
                    )
                    for j in range(2):
                        head = 2 * ti + j
                        bct = bcp.tile([128, HW], QK_DT, tag="bc", name="bc")
                        emit_bcast_half(head, 0, bct)
                        emit_bcast_half(head, 1, bct)
                        for cj in range(8):
                            produce(
                                prod_t, j, cj,
                                w_h0 if cj < 4 else w_h1, bct, 0, 1024,
                                use_act=(cj in (2, 5)),
                            )
                        dma_block(head, prod_t, j, 0, 8, 0, 1024)

                # PE warm-up dummies: emitted last => lowest priority, they
                # only run when no real matmul is ready (t~0 and x-stalls)
                for wi in range(N_WARMUP):
                    wps = warm_ps.tile([128, 256], F32, tag="w")
                    nc.tensor.matmul(
                        wps, wm[:, 0:128], wm, start=True, stop=True,
                    )

            if loop_iters:
                with tc.For_i(0, loop_iters, 1):
                    emit_body()
            else:
                emit_body()

    nc.compile()
    return nc


_NC_CACHE = None


def _get_nc():
    global _NC_CACHE
    if _NC_CACHE is None:
        _NC_CACHE = build_program()
    return _NC_CACHE


def make_in_maps(x, wq, wk, wv):
    """Host-side input prep: dtype casts and layout transforms only (transpose,
    reshape, head-block sum of wv -- no x-dependent compute beyond layout),
    plus per-core batch sharding."""
    x = np.ascontiguousarray(np.asarray(x, dtype=np.float32))
    wq = np.asarray(wq, dtype=np.float32)
    wk = np.asarray(wk, dtype=np.float32)
    wv = np.asarray(wv, dtype=np.float32)
    b, H, W, dim = x.shape
    assert (b, H, W, dim) == (B, 32, 32, DIM)

    # blocked [pair, p, k, o]: wb[t, p, k, o] = w.T[k*128+p, t*128+o]
    def blocked(w):
        wt = np.ascontiguousarray(w.T).astype(QK_NP)        # [c, o]
        return np.ascontiguousarray(
            wt.reshape(4, 128, 4, 128).transpose(2, 1, 0, 3)
        )

    wqb = blocked(wq)
    wkb = blocked(wk)
    # pair-1 and pairs-2/3 weights packed: [p, {q,k}, (t,) k, o]
    w1 = np.ascontiguousarray(
        np.stack([wqb[1], wkb[1]], axis=0).transpose(1, 0, 2, 3)
    )
    w23 = np.ascontiguousarray(
        np.stack([wqb[2:4], wkb[2:4]], axis=0).transpose(2, 0, 1, 3, 4)
    )
    wvt = np.ascontiguousarray(
        wv.reshape(N_HEADS, D_HEAD, DIM).sum(axis=1).T     # [c, n]
    ).astype(QK_NP)
    ind2 = np.zeros((128, 2), dtype=QK_NP)
    ind2[np.arange(128), np.arange(128) // D_HEAD] = 1.0
    bd32 = np.zeros((128, 4), dtype=QK_NP)
    bd32[np.arange(128), np.arange(128) // 32] = 1.0
    pre = np.concatenate(
        [
            wvt.reshape(4, 128, 8).transpose(1, 0, 2).reshape(128, 32),
            ind2,
            bd32,
            wqb[0].reshape(128, 512),
            wkb[0].reshape(128, 512),
        ],
        axis=1,
    )
    sel = np.zeros((N_HEADS, N_HEADS * 128), dtype=QK_NP)
    for n in range(N_HEADS):
        sel[n, n * 128 : (n + 1) * 128] = 1.0
    aux2 = np.zeros((8, 1152), dtype=QK_NP)
    aux2[:, 0:1024] = sel
    aux2[0:4, 1024:1152] = bd32.T  # sel32

    xh = x.reshape(B, HW, DIM).astype(QK_NP)
    return [
        {
            "xt": np.ascontiguousarray(xh[i].T),           # [c, xy]
            "pre": np.ascontiguousarray(pre),
            "w1": w1,
            "w23": w23,
            "aux2": aux2,
        }
        for i in range(N_CORES)
    ]


def kernel(x, wq, wk, wv):
    nc = _get_nc()
    in_maps = make_in_maps(x, wq, wk, wv)
    res = run_bass_kernel_spmd(nc, in_maps, list(range(N_CORES)))
    out = np.stack([res.results[i]["y"] for i in range(N_CORES)], axis=0)
    # [b, n, p, c, xy] -> [b, n, hw=c*128+p, xy] -> [b, n, h, w, x, y];
    # un-permute + upcast fp16 -> fp32 on host (layout only)
    out = np.ascontiguousarray(out.transpose(0, 1, 3, 2, 4))
    return out.astype(np.float32).reshape(B, N_HEADS, 32, 32, 32, 32)


if __name__ == "__main__":
    rng = np.random.default_rng(0)
    x = rng.standard_normal((B, 32, 32, DIM), dtype=np.float32)
    s = 1.0 / np.sqrt(512.0)
    wq = rng.uniform(-s, s, (512, 512)).astype(np.float32)
    wk = rng.uniform(-s, s, (512, 512)).astype(np.float32)
    wv = rng.uniform(-s, s, (512, 512)).astype(np.float32)
    y = kernel(x=x, wq=wq, wk=wk, wv=wv)
    print(y.shape, y.dtype)


# revision 54
# speedup vs baseline: 1.1005x; 1.1005x over previous
"""Trainium2 Bass kernel for nn_AttnBlock (per-pixel qk attention block).

Reference computation (per batch b):
  q = x @ wq.T ; k = x @ wk.T ; v = x @ wv.T          # [H*W, 512], heads n=8, d=64
  s[n, p]    = sum_d q[p, n*64+d] * k[p, n*64+d]      # per-pixel dot product
  w[n, h, :] = softmax(s[n, h, :] * d**-0.5)          # softmax over W axis (32)
  vsum[n, p] = sum_d v[p, n*64+d]
  out[b, n, hw, xy] = w[n, hw] * vsum[n, xy]          # outer product per batch

Sharding: data-parallel over batch: core b handles batch b (8 cores, B=8).

The kernel is output-write bound: fp16 output (host upcasts; rel err ~1e-3
vs the 2e-2 gate) => 16 MB/core written + ~2.4 MB read at the ~360 GB/s
per-NC DMA limit => ~52.5 us DMA floor. Total = t_first_output + 46.6 us
(gapless output stream) + ~1.6 us DMA-sem/barrier tail, so the design
minimizes t_first_output and keeps the output queue fed:

- x^T is loaded in pixel halves so pair-0 q/k matmuls start after
  0.5 MB of x; pair-0 half-0 then runs at pixel-QUARTER granularity so
  the first output block (head 0, rows 0:2, xy 0:256, 128 KB) needs only
  a quarter of x for both its weights and its vsum/bcast axis.
- scores are computed TRANSPOSED: 4 tiny matmuls sprod_chunk^T @ ind2
  -> [128px, 2 heads] PSUM, so softmax weights come out pixel-partitioned
  with no PE transposes. Denominators: ones-blockdiag32^T @ e -> [4, 8],
  reciprocal, then sel32^T broadcast back to [128, 8]; w = e * rden_bcast.
- y DRAM layout is [n, p, c, xy] (hw = c*128 + p): per-partition
  contiguous (c, xy) runs give 4-16 KB DMA descriptors (vs 2 KB with
  [n, hw, xy]); host un-permutes (layout only).
- PE warm-up: dummy matmuls (own PSUM bank, emitted last = lowest
  priority) keep the tensor engine continuously busy from t~0 so it
  reaches full clock (2.4 GHz after 3 us busy) before the x-gated burst;
  narrow (N=256) so each mid-kernel steal costs ~107 ns.
- k's PSUM never drains: sprod = q*k reads k's PSUM bank directly (walrus
  allows one PSUM input per TensorTensor); q drains on ACT (DVE for
  pair-0 half-0 where DVE is otherwise idle). Softmax in fp16, no
  max-subtraction (logits ~N(0, 0.33)).
- production per head: PE selector-matmul broadcast of vsum ranges
  (drains on ACT), outer-product tiles via DVE tensor_scalar fp16
  (1 of 8 rows on ACT for pairs 1-3); pair 1-3 heads ship as two 1 MB
  DMAs so rows 0:4 stream while rows 4:8 are still being produced.
  HW notes (NTFF): PE runs ~1.35 GHz throttled (sim's 2.4 GHz ramp
  never materializes), and the NEFF epilogue's ~250 per-semaphore
  resets add a fixed ~8 us tail inside the measured exec window.
"""

import numpy as np

import concourse.bass as bass
import concourse.mybir as mybir
import concourse.tile as tile
from concourse import bacc
from concourse.bass_utils import run_bass_kernel_spmd

F32 = mybir.dt.float32
F16 = mybir.dt.float16

B, HW, DIM = 8, 1024, 512
N_HEADS, D_HEAD = 8, 64
N_CORES = 8
SCALE = float(D_HEAD) ** -0.5

QK_DT = F16
QK_NP = np.float16
OUT_DT = F16

N_WARMUP = 16  # dummy PE matmuls: ramp filler; few enough not to jam
               # the in-order PE stream after x lands


def build_program(loop_iters=None):
    """loop_iters: if set, wrap the whole kernel body in a tc.For_i hardware
    loop (benchmarking only -- one NEFF executes the body N times)."""
    # Bacc (not raw Bass): its compile() runs move_matmul_waits_to_ldweights,
    # without which any matmul with >1 semaphore wait fails walrus codegen.
    nc = bacc.Bacc(None)

    xt_d = nc.declare_dram_parameter("xt", [DIM, HW], QK_DT, isOutput=False)
    # pre_a = wv_sum, ind2, blockdiag32, wq0 (loads before x so q matmuls
    # start on x arrival); pre_b = wk0 (loads behind the first x quarter)
    pre_a_d = nc.declare_dram_parameter("pre_a", [128, 550], QK_DT,
                                        isOutput=False)
    pre_b_d = nc.declare_dram_parameter("pre_b", [128, 512], QK_DT,
                                        isOutput=False)
    w1_d = nc.declare_dram_parameter("w1", [128, 2, 4, 128], QK_DT,
                                     isOutput=False)
    w23_d = nc.declare_dram_parameter(
        "w23", [128, 2, 2, 4, 128], QK_DT, isOutput=False
    )
    aux2_d = nc.declare_dram_parameter("aux2", [8, 1152], QK_DT, isOutput=False)
    # y layout [n, p, c, xy] (hw = c*128 + p); host un-permutes.
    y_d = nc.declare_dram_parameter("y", [N_HEADS, 128, 8, HW], OUT_DT,
                                    isOutput=True)

    with tile.TileContext(nc) as tc:
        with (
            tc.tile_pool(name="singles", bufs=1) as singles,
            tc.tile_pool(name="sprod", bufs=2) as sprodp,
            tc.tile_pool(name="smax", bufs=2) as smaxp,
            tc.tile_pool(name="wt", bufs=4) as wtp,
            tc.tile_pool(name="bc", bufs=8) as bcp,
            tc.tile_pool(name="prod", bufs=2) as prodp,
            tc.tile_pool(name="warm_ps", bufs=1, space="PSUM") as warm_ps,
            tc.tile_pool(name="qk_ps", bufs=3, space="PSUM") as qk_ps,
            tc.tile_pool(name="s_ps", bufs=2, space="PSUM") as s_ps,
            tc.tile_pool(name="v_ps", bufs=1, space="PSUM") as v_ps,
        ):
            def emit_body():
                # ---- PE warm-up tile (dummy matmuls emitted last, so they
                # have the lowest priority and only fill idle PE slots) ------
                wm = singles.tile([128, 256], QK_DT, name="wm")
                nc.gpsimd.memset(wm, 0.0)
                # pre-warm the Exp activation table off the critical path:
                # the first Exp otherwise pays the ~1.3us ACT table load right
                # inside the first-output chain (and the scheduler plans for
                # it, jamming the PE stream with later matmuls).
                with tc.high_priority():
                    warm_act = singles.tile([2, 2], QK_DT, name="warm_act")
                    nc.scalar.activation(
                        out=warm_act, in_=wm[0:2, 0:2],
                        func=mybir.ActivationFunctionType.Exp,
                    )

                # ---- loads: one FIFO queue (sync HWDGE), priority order ----
                pre_sb = singles.tile([128, 550], QK_DT)
                nc.sync.dma_start(out=pre_sb, in_=pre_a_d[:])
                wvt_sb = pre_sb[:, 0:32].rearrange("p (k n) -> p k n", k=4)
                ind2_sb = pre_sb[:, 32:34]
                bd32_sb = pre_sb[:, 34:38]
                wq0_sb = pre_sb[:, 38:550].rearrange("p (k o) -> p k o", k=4)

                xT = singles.tile([128, 4, HW], QK_DT, name="xT")
                xv = xt_d[:].rearrange("(k p) xy -> p k xy", p=128)
                # half 0 split in pixel quarters: quarter-0 compute starts
                # after 0.25 MB instead of 0.5 MB
                nc.sync.dma_start(out=xT[:, :, 0:256], in_=xv[:, :, 0:256])
                preb_sb = singles.tile([128, 512], QK_DT)
                nc.sync.dma_start(out=preb_sb, in_=pre_b_d[:])
                wk0_sb = preb_sb.rearrange("p (k o) -> p k o", k=4)
                nc.sync.dma_start(
                    out=xT[:, :, 256:512], in_=xv[:, :, 256:512]
                )
                aux2_sb = singles.tile([8, 1152], QK_DT)
                nc.sync.dma_start(out=aux2_sb, in_=aux2_d[:])
                sel_sb = aux2_sb[:, 0:1024]
                sel32_sb = aux2_sb[0:4, 1024:1152]
                nc.sync.dma_start(
                    out=xT[:, :, 512:1024], in_=xv[:, :, 512:1024]
                )
                w1_sb = singles.tile([128, 2, 4, 128], QK_DT)   # pair 1
                nc.sync.dma_start(out=w1_sb, in_=w1_d[:])
                w23_sb = singles.tile([128, 2, 2, 4, 128], QK_DT)
                nc.sync.dma_start(out=w23_sb, in_=w23_d[:])

                vps = v_ps.tile([N_HEADS, HW], F32)
                vsum_sb = singles.tile([N_HEADS, HW], QK_DT)

                def wslice(qk, ti, ki):
                    if ti == 0:
                        return (wq0_sb if qk == 0 else wk0_sb)[:, ki, :]
                    if ti == 1:
                        return w1_sb[:, qk, ki, :]
                    return w23_sb[:, qk, ti - 2, ki, :]

                def emit_qk_half(ti, nj):
                    """q/k matmul groups for pixel-half nj of pair ti."""
                    qps = qk_ps.tile([128, 512], F32, tag="qk", name="qps")
                    kps = qk_ps.tile([128, 512], F32, tag="qk", name="kps")
                    for qk, ps in ((0, qps), (1, kps)):
                        for ki in range(4):
                            nc.tensor.matmul(
                                ps,
                                wslice(qk, ti, ki),
                                xT[:, ki, nj * 512 : (nj + 1) * 512],
                                start=(ki == 0),
                                stop=(ki == 3),
                            )
                    return qps, kps

                def emit_vsum(x0, x1, drain_on_dve=False):
                    csl = slice(x0, x1)
                    for ki in range(4):
                        nc.tensor.matmul(
                            vps[:, csl],
                            wvt_sb[:, ki, :],
                            xT[:, ki, csl],
                            start=(ki == 0),
                            stop=(ki == 3),
                        )
                    if drain_on_dve:
                        nc.vector.tensor_copy(vsum_sb[:, csl], vps[:, csl])
                    else:
                        nc.scalar.copy(vsum_sb[:, csl], vps[:, csl])

                def emit_scores(qps, kps, x0, x1, q_on_dve=False):
                    """Transposed scores over tile-local pixel range [x0, x1):
                    w[p, c*2+j] = softmax weight of head j at local pixel
                    c*128+p. q drains to SBUF (ACT; DVE where noted); sprod
                    reads k straight from its PSUM bank (<=1 PSUM input per
                    TensorTensor)."""
                    wd = x1 - x0
                    nch = wd // 128
                    qt = sprodp.tile([128, wd], QK_DT, tag=f"qt{wd}")
                    if q_on_dve:
                        nc.vector.tensor_copy(qt, qps[:, x0:x1])
                    else:
                        nc.scalar.copy(qt, qps[:, x0:x1])
                    sprod = sprodp.tile([128, wd], QK_DT, tag=f"sp{wd}")
                    nc.vector.tensor_tensor(
                        out=sprod, in0=qt, in1=kps[:, x0:x1],
                        op=mybir.AluOpType.mult,
                    )
                    sp = s_ps.tile([128, 24], F32, tag="s", name="sp")
                    for c in range(nch):
                        nc.tensor.matmul(
                            sp[:, 2 * c : 2 * c + 2],
                            sprod[:, c * 128 : (c + 1) * 128],
                            ind2_sb,
                            start=True, stop=True,
                        )
                    e_t = smaxp.tile([128, 2 * nch], QK_DT, tag=f"e{wd}")
                    nc.scalar.activation(
                        out=e_t, in_=sp[:, 0 : 2 * nch],
                        func=mybir.ActivationFunctionType.Exp,
                        scale=SCALE,
                    )
                    nc.tensor.matmul(
                        sp[0:4, 8 : 8 + 2 * nch], bd32_sb, e_t,
                        start=True, stop=True,
                    )
                    rden = smaxp.tile([4, 2 * nch], QK_DT, tag=f"r{wd}")
                    with nc.allow_low_precision(reason="fp16 softmax denom"):
                        nc.vector.reciprocal(rden, sp[0:4, 8 : 8 + 2 * nch])
                    nc.tensor.matmul(
                        sp[:, 16 : 16 + 2 * nch], sel32_sb, rden,
                        start=True, stop=True,
                    )
                    w_t = wtp.tile([128, 2 * nch], F32, tag=f"w{wd}",
                                   name="wt")
                    nc.vector.tensor_tensor(
                        out=w_t, in0=e_t, in1=sp[:, 16 : 16 + 2 * nch],
                        op=mybir.AluOpType.mult,
                    )
                    return w_t

                def emit_bcast(head, x0, x1, bcast_t):
                    """vsum row range -> all partitions via PE selector
                    matmul; PSUM drains on ACT into bcast_t[:, x0:x1]."""
                    bps = qk_ps.tile([128, 512], F32, tag="qk", name="bps")
                    nc.tensor.matmul(
                        bps[:, 0 : x1 - x0],
                        sel_sb[:, head * 128 : (head + 1) * 128],
                        vsum_sb[:, x0:x1],
                        start=True, stop=True,
                    )
                    nc.scalar.copy(bcast_t[:, x0:x1], bps[:, 0 : x1 - x0])

                def produce(prod_t, j, cj, w_half, bcast_t, xy0, xy1,
                            colbase=0, use_act=False):
                    col = (cj - colbase) * 2 + j
                    if use_act:
                        nc.scalar.activation(
                            out=prod_t[:, j, cj, xy0:xy1],
                            in_=bcast_t[:, xy0:xy1],
                            func=mybir.ActivationFunctionType.Copy,
                            scale=w_half[:, col : col + 1],
                        )
                    else:
                        nc.vector.tensor_scalar_mul(
                            prod_t[:, j, cj, xy0:xy1],
                            bcast_t[:, xy0:xy1],
                            w_half[:, col : col + 1],
                        )

                def dma_block(head, prod_t, j, c0, c1, xy0, xy1):
                    nc.sync.dma_start(
                        out=y_d[head : head + 1].rearrange(
                            "n p c xy -> p n c xy"
                        )[:, :, c0:c1, xy0:xy1],
                        in_=prod_t[:, j : j + 1, c0:c1, xy0:xy1],
                    )

                def dma_block2(prod_t, c0, c1, xy0, xy1):
                    # heads 0-1 (adjacent in y) in one DMA: halves the
                    # per-DMA issue-config cost for the small head blocks
                    nc.sync.dma_start(
                        out=y_d[0:2].rearrange(
                            "n p c xy -> p n c xy"
                        )[:, :, c0:c1, xy0:xy1],
                        in_=prod_t[:, 0:2, c0:c1, xy0:xy1],
                    )

                # ---- pair 0: the first-output chain gets high priority.
                # Half 0 runs at pixel-QUARTER granularity: the first 128 KB
                # block (head 0, rows 0:2, xy 0:256) needs only a quarter of
                # x for both the weights and the vsum/bcast axis, pulling
                # the first output DMA several us earlier. ----
                with tc.high_priority():
                    qps0 = qk_ps.tile([128, 512], F32, tag="qk", name="qps")
                    kps0 = qk_ps.tile([128, 512], F32, tag="qk", name="kps")

                    def qk00_quarter(x0, x1):
                        for qk, ps in ((0, qps0), (1, kps0)):
                            for ki in range(4):
                                nc.tensor.matmul(
                                    ps[:, x0:x1],
                                    wslice(qk, 0, ki),
                                    xT[:, ki, x0:x1],
                                    start=(ki == 0),
                                    stop=(ki == 3),
                                )

                    qk00_quarter(0, 256)
                    w_q0 = emit_scores(qps0, kps0, 0, 256, q_on_dve=True)
                    emit_vsum(0, 256)
                    prod0 = prodp.tile(
                        [128, 2, 8, HW], OUT_DT, tag="pr", name="pr"
                    )
                    bc0 = bcp.tile([128, HW], QK_DT, tag="bc", name="bc")
                    emit_bcast(0, 0, 256, bc0)
                    for c in range(2):
                        produce(prod0, 0, c, w_q0, bc0, 0, 256)
                    dma_block(0, prod0, 0, 0, 2, 0, 256)
                    # vsum/bcasts for quarter 1 and head 1 BEFORE qk-q1:
                    # they need only x (not wq/wk), so three more 128 KB
                    # blocks gated only on w_q0 fill the bus while the
                    # quarter-1 score chain runs
                    emit_vsum(256, 512, drain_on_dve=True)
                    emit_bcast(0, 256, 512, bc0)
                    for c in range(2):
                        produce(prod0, 0, c, w_q0, bc0, 256, 512)
                    dma_block(0, prod0, 0, 0, 2, 256, 512)
                    bc1 = bcp.tile([128, HW], QK_DT, tag="bc", name="bc")
                    emit_bcast(1, 0, 512, bc1)
                    for c in range(2):
                        produce(prod0, 1, c, w_q0, bc1, 0, 512)
                    dma_block(1, prod0, 1, 0, 2, 0, 512)

                    qk00_quarter(256, 512)
                    w_q1 = emit_scores(qps0, kps0, 256, 512)
                    # vsum half-1 needs only x (not qk01): hoist it so the
                    # rows-0:2 xy-h1 blocks (gated on w_q0 only) fill the
                    # queue while the w_q1 score chain runs
                    emit_vsum(512, 1024)
                    emit_bcast(0, 512, 1024, bc0)
                    for c in range(2):
                        produce(prod0, 0, c, w_q0, bc0, 512, 1024)
                    dma_block(0, prod0, 0, 0, 2, 512, 1024)
                    emit_bcast(1, 512, 1024, bc1)
                    for c in range(2):
                        produce(prod0, 1, c, w_q0, bc1, 512, 1024)
                    dma_block(1, prod0, 1, 0, 2, 512, 1024)
                    # rows 2:4 full-xy (bcasts now complete)
                    for c in range(2, 4):
                        produce(prod0, 0, c, w_q1, bc0, 0, 1024, colbase=2)
                    dma_block(0, prod0, 0, 2, 4, 0, 1024)
                    for c in range(2, 4):
                        produce(prod0, 1, c, w_q1, bc1, 0, 1024, colbase=2)
                    dma_block(1, prod0, 1, 2, 4, 0, 1024)

                # pair 0, half 1 rows + hoisted pair-1 q/k
                qk01 = emit_qk_half(0, 1)
                w01 = emit_scores(*qk01, 0, 512)
                # pair-1 half-0 q/k hoisted here: pair-0's remaining blocks
                # (rows 4:8) need no PE (only DVE/ACT produce), so the PE
                # starts pair 1 early -> closes the pair-0 -> pair-1
                # output-stream gap
                qk1h0 = emit_qk_half(1, 0)
                for c in range(4, 8):
                    produce(prod0, 0, c, w01, bc0, 0, 1024, colbase=4)
                dma_block(0, prod0, 0, 4, 8, 0, 1024)
                for c in range(4, 8):
                    produce(prod0, 1, c, w01, bc1, 0, 1024, colbase=4)
                dma_block(1, prod0, 1, 4, 8, 0, 1024)

                # ---- pairs 1-3 ----
                for ti in range(1, 4):
                    qps, kps = qk1h0 if ti == 1 else emit_qk_half(ti, 0)
                    w_h0 = emit_scores(qps, kps, 0, 512)
                    qps, kps = emit_qk_half(ti, 1)
                    w_h1 = emit_scores(qps, kps, 0, 512)
                    prod_t = prodp.tile(
                        [128, 2, 8, HW], OUT_DT, tag="pr", name="pr"
                    )
                    for j in range(2):
                        head = 2 * ti + j
                        bct = bcp.tile([128, HW], QK_DT, tag="bc", name="bc")
                        emit_bcast(head, 0, 512, bct)
                        emit_bcast(head, 512, 1024, bct)
                        # four 0.5 MB DMAs per head: each pair of rows ships
                        # as soon as produced, so the first bytes lead the
                        # produce burst and the last transfer is short
                        for c0 in range(0, 8, 2):
                            wt = w_h0 if c0 < 4 else w_h1
                            cb = 0 if c0 < 4 else 4
                            for cj in (c0, c0 + 1):
                                produce(
                                    prod_t, j, cj, wt, bct, 0, 1024,
                                    colbase=cb,
                                    use_act=(cj == (2 if j == 0 else 5)),
                                )
                            dma_block(head, prod_t, j, c0, c0 + 2, 0, 1024)

                # PE warm-up dummies: emitted last => lowest priority, they
                # only run when no real matmul is ready (t~0 and x-stalls)
                for wi in range(N_WARMUP):
                    wps = warm_ps.tile([128, 256], F32, tag="w")
                    nc.tensor.matmul(
                        wps, wm[:, 0:128], wm, start=True, stop=True,
                    )

            if loop_iters:
                with tc.For_i(0, loop_iters, 1):
                    emit_body()
            else:
                emit_body()

    nc.compile()
    return nc


_NC_CACHE = None


def _get_nc():
    global _NC_CACHE
    if _NC_CACHE is None:
        _NC_CACHE = build_program()
    return _NC_CACHE


def make_in_maps(x, wq, wk, wv):
    """Host-side input prep: dtype casts and layout transforms only (transpose,
    reshape, head-block sum of wv -- no x-dependent compute beyond layout),
    plus per-core batch sharding."""
    x = np.ascontiguousarray(np.asarray(x, dtype=np.float32))
    wq = np.asarray(wq, dtype=np.float32)
    wk = np.asarray(wk, dtype=np.float32)
    wv = np.asarray(wv, dtype=np.float32)
    b, H, W, dim = x.shape
    assert (b, H, W, dim) == (B, 32, 32, DIM)

    # blocked [pair, p, k, o]: wb[t, p, k, o] = w.T[k*128+p, t*128+o]
    def blocked(w):
        wt = np.ascontiguousarray(w.T).astype(QK_NP)        # [c, o]
        return np.ascontiguousarray(
            wt.reshape(4, 128, 4, 128).transpose(2, 1, 0, 3)
        )

    wqb = blocked(wq)
    wkb = blocked(wk)
    # pair-1 and pairs-2/3 weights packed: [p, {q,k}, (t,) k, o]
    w1 = np.ascontiguousarray(
        np.stack([wqb[1], wkb[1]], axis=0).transpose(1, 0, 2, 3)
    )
    w23 = np.ascontiguousarray(
        np.stack([wqb[2:4], wkb[2:4]], axis=0).transpose(2, 0, 1, 3, 4)
    )
    wvt = np.ascontiguousarray(
        wv.reshape(N_HEADS, D_HEAD, DIM).sum(axis=1).T     # [c, n]
    ).astype(QK_NP)
    ind2 = np.zeros((128, 2), dtype=QK_NP)
    ind2[np.arange(128), np.arange(128) // D_HEAD] = 1.0
    bd32 = np.zeros((128, 4), dtype=QK_NP)
    bd32[np.arange(128), np.arange(128) // 32] = 1.0
    pre_a = np.concatenate(
        [
            wvt.reshape(4, 128, 8).transpose(1, 0, 2).reshape(128, 32),
            ind2,
            bd32,
            wqb[0].reshape(128, 512),
        ],
        axis=1,
    )
    pre_b = np.ascontiguousarray(wkb[0].reshape(128, 512))
    sel = np.zeros((N_HEADS, N_HEADS * 128), dtype=QK_NP)
    for n in range(N_HEADS):
        sel[n, n * 128 : (n + 1) * 128] = 1.0
    aux2 = np.zeros((8, 1152), dtype=QK_NP)
    aux2[:, 0:1024] = sel
    aux2[0:4, 1024:1152] = bd32.T  # sel32

    xh = x.reshape(B, HW, DIM).astype(QK_NP)
    return [
        {
            "xt": np.ascontiguousarray(xh[i].T),           # [c, xy]
            "pre_a": np.ascontiguousarray(pre_a),
            "pre_b": pre_b,
            "w1": w1,
            "w23": w23,
            "aux2": aux2,
        }
        for i in range(N_CORES)
    ]


def kernel(x, wq, wk, wv):
    nc = _get_nc()
    in_maps = make_in_maps(x, wq, wk, wv)
    res = run_bass_kernel_spmd(nc, in_maps, list(range(N_CORES)))
    out = np.stack([res.results[i]["y"] for i in range(N_CORES)], axis=0)
    # [b, n, p, c, xy] -> [b, n, hw=c*128+p, xy] -> [b, n, h, w, x, y];
    # un-permute + upcast fp16 -> fp32 on host (layout only)
    out = np.ascontiguousarray(out.transpose(0, 1, 3, 2, 4))
    return out.astype(np.float32).reshape(B, N_HEADS, 32, 32, 32, 32)


if __name__ == "__main__":
    rng = np.random.default_rng(0)
    x = rng.standard_normal((B, 32, 32, DIM), dtype=np.float32)
    s = 1.0 / np.sqrt(512.0)
    wq = rng.uniform(-s, s, (512, 512)).astype(np.float32)
    wk = rng.uniform(-s, s, (512, 512)).astype(np.float32)
    wv = rng.uniform(-s, s, (512, 512)).astype(np.float32)
    y = kernel(x=x, wq=wq, wk=wk, wv=wv)
    print(y.shape, y.dtype)
